# revision 5
# baseline (speedup 1.0000x reference)
"""CapsuleModel2 segment-reduce kernel for 8 TRN2 NeuronCores.

Math (per reference.py):
    feats  = class_capsules.reshape(65536, 272)[point_idx]        # [P, 272]
    sums   = segment_sum(feats, segment_ids, 4096)                # [4096, 272]
    counts = segment_sum(ones)                                    # [4096]
    out    = sigmoid((sums / max(counts,1)) @ W + b)              # [4096, 19]

Key identity used: (sums @ W) = segment_sum(feats @ W), so we project the
65536x272 grid down to 19 channels FIRST (on the PE), then gather 20-wide
rows (19 channels + a constant-1 count column) instead of 272-wide rows.

Distribution (table-sharded):
  - core k owns grid rows [k*8192, (k+1)*8192) and the points that hit them
  - each core computes partial sums over ALL 4096 segments via a one-hot
    matmul (exploits segment-sortedness: 128 consecutive points of a core
    stay inside one 128-segment block after per-block padding)
  - ReduceScatter(add) gives core k the totals for segments [k*512,(k+1)*512)
  - finalize: divide by counts, add bias, sigmoid

Pipeline per core:
  A) proj:   table[8192, 128]bf16 = gridT.T @ [W | 1] (cols 20..127 zero)
  B) gather: dma_gather point rows (padded per 128-seg block, idx 0 = pad)
     reduce: psum[128seg, 20] += onehot(segrel)^T @ X  (18 chunks per block)
  C) ReduceScatter + finalize + write [512, 19]
"""

import sys

for _p in ('/opt/trn_rl_repo',):
    if _p not in sys.path:
        sys.path.insert(0, _p)

import numpy as np
import ml_dtypes

import concourse.bacc as bacc
import concourse.bass as bass
import concourse.mybir as mybir
import concourse.tile as tile

BF16 = mybir.dt.bfloat16
F32 = mybir.dt.float32
I16 = mybir.dt.int16
F16 = mybir.dt.float16

NCORE = 8
GRID = 65536
GPC = GRID // NCORE          # 8192 grid rows per core
D = 272                      # capsule feature dim
NCH = 19                     # output channels
NW = NCH + 1                 # + count column
NSEG = 4096
SEGBLK = 128                 # segments per block
NBLK = NSEG // SEGBLK        # 32
CAP = 2304                   # padded points per (core, block); actual max 2181
CPB = CAP // 128             # 18 chunks per block
NCHUNK = NBLK * CPB          # 576 chunks per core
NIDX = NCHUNK * 128          # 73728 gather slots per core
NSLICE = 24
CHUNKS_PER_SLICE = NCHUNK // NSLICE   # 72
IDX_PER_SLICE = NIDX // NSLICE        # 9216
BLKS_PER_SLICE = NBLK // NSLICE       # 4
ELEM = 128                   # table row width (bf16) = 256B
SEG_PER_CORE = NSEG // NCORE  # 512
MTILE = 512                  # grid columns loaded per projection step


def build_nc(skip_collective=False, rs_inloop=False):
    nc = bacc.Bacc("TRN2", num_devices=NCORE, num_swdge_queues=4)

    gridT = nc.dram_tensor("gridT", [D, GPC], BF16, kind="ExternalInput")
    w_pack = nc.dram_tensor("w_pack", [128, 60], BF16, kind="ExternalInput")
    idx_in = nc.dram_tensor("idx", [NSLICE, 128, IDX_PER_SLICE // 16], I16,
                            kind="ExternalInput")
    segrel_in = nc.dram_tensor("segrel", [128, NCHUNK], BF16, kind="ExternalInput")
    iota_in = nc.dram_tensor("iota", [128, SEGBLK], BF16, kind="ExternalInput")
    bias_in = nc.dram_tensor("bias", [128, NCH], F32, kind="ExternalInput")
    if skip_collective:
        out_t = nc.dram_tensor("out", [NBLK, 128, NW], F16, kind="ExternalOutput")
    else:
        out_t = nc.dram_tensor("out", [SEG_PER_CORE, NCH], F32, kind="ExternalOutput")

    table = nc.dram_tensor("table", [GPC, ELEM], BF16)
    partial_d = nc.dram_tensor("partial", [NBLK, 128, NW], F16)
    rs_out = nc.dram_tensor("rs_out", [8, 64, NW], F16)

    with tile.TileContext(nc) as tc:
        with (
            tc.tile_pool(name="const", bufs=1) as cpool,
            tc.tile_pool(name="grid", bufs=3) as gpool,
            tc.tile_pool(name="ptab", bufs=2, space="PSUM") as pt_pool,
            tc.tile_pool(name="tab", bufs=4) as tpool,
            tc.tile_pool(name="idxp", bufs=1) as ipool,
            tc.tile_pool(name="dst", bufs=6) as dpool,
            tc.tile_pool(name="oh", bufs=6) as opool,
            tc.tile_pool(name="pblk", bufs=6, space="PSUM") as pb_pool,
            tc.tile_pool(name="acc", bufs=1) as apool,
            tc.tile_pool(name="fin", bufs=2) as fpool,
        ):
            # constants
            w_sb = cpool.tile([128, 60], BF16)
            nc.sync.dma_start(w_sb[:], w_pack[:])
            segrel_sb = cpool.tile([128, NCHUNK], BF16)
            nc.sync.dma_start(segrel_sb[:], segrel_in[:])
            iota_sb = cpool.tile([128, SEGBLK], BF16)
            nc.sync.dma_start(iota_sb[:], iota_in[:])
            bias_sb = cpool.tile([128, NCH], F32)
            nc.sync.dma_start(bias_sb[:], bias_in[:])

            # ---- Phase A: projection -> table ----
            ksizes = [(0, 128), (128, 128), (256, 16)]
            for mt in range(GPC // MTILE):
                gt = gpool.tile([128, 3, MTILE], BF16, tag="gt")
                for t, (k0, kn) in enumerate(ksizes):
                    nc.sync.dma_start(
                        gt[:kn, t, :],
                        gridT[k0:k0 + kn, mt * MTILE:(mt + 1) * MTILE])
                tab = tpool.tile([128, MTILE // 128, ELEM], BF16, tag="tab")
                for ms in range(MTILE // 128):
                    psum = pt_pool.tile([128, NW], F32, tag="ptab")
                    for t, (k0, kn) in enumerate(ksizes):
                        nc.tensor.matmul(
                            out=psum[:],
                            lhsT=gt[:kn, t, ms * 128:(ms + 1) * 128],
                            rhs=w_sb[:kn, t * NW:(t + 1) * NW],
                            start=(t == 0), stop=(t == 2))
                    nc.vector.memset(tab[:, ms, NCH:], 0.0)
                    nc.vector.memset(tab[:, ms, NCH:NW], 1.0)
                    nc.vector.tensor_copy(tab[:, ms, :NCH], psum[:, :NCH])
                nc.scalar.dma_start(
                    table[mt * MTILE:(mt + 1) * MTILE]
                        .rearrange("(ms p) e -> p ms e", p=128),
                    tab[:])

            # ---- Phase B: gather + one-hot reduce ----
            part_sb = apool.tile([128, NBLK * NW], F16)
            idx_all = ipool.tile([128, NSLICE, IDX_PER_SLICE // 16], I16)
            psum_b = None
            nc.sync.dma_start(idx_all[:], idx_in[:].rearrange("s p c -> p s c"))
            for s in range(NSLICE):
                idx_sb = idx_all[:, s, :]
                dst = dpool.tile([128, CHUNKS_PER_SLICE, ELEM], BF16, tag="dst")
                nc.gpsimd.dma_gather(
                    dst[:], table[:], idx_sb[:],
                    IDX_PER_SLICE, IDX_PER_SLICE, ELEM, single_packet=False,
                    queue_num=s % 4)
                oh = opool.tile([128, CHUNKS_PER_SLICE, SEGBLK], BF16, tag="oh")
                nc.vector.tensor_tensor(
                    out=oh[:],
                    in0=segrel_sb[:, s * CHUNKS_PER_SLICE:(s + 1) * CHUNKS_PER_SLICE]
                        .rearrange("p (c o) -> p c o", o=1).broadcast_to(
                            [128, CHUNKS_PER_SLICE, SEGBLK]),
                    in1=iota_sb[:].rearrange("p (o j) -> p o j", o=1).broadcast_to(
                            [128, CHUNKS_PER_SLICE, SEGBLK]),
                    op=mybir.AluOpType.is_equal)
                for g in range(CHUNKS_PER_SLICE):
                    gg = s * CHUNKS_PER_SLICE + g
                    blk, j = divmod(gg, CPB)
                    if j == 0:
                        psum_b = pb_pool.tile([128, NW], F32, tag="pblk")
                    nc.tensor.matmul(
                        out=psum_b[:],
                        lhsT=oh[:, g, :],
                        rhs=dst[:, g, :NW],
                        start=(j == 0), stop=(j == CPB - 1))
                    if j == CPB - 1:
                        nc.scalar.copy(
                            part_sb[:, blk * NW:(blk + 1) * NW], psum_b[:])
                # each 3 slices complete one 4-block RS chunk; issue the
                # store right away, but stagger the collective trigger two
                # gather-gens later so the Pool engine never stalls on it
                if s % 3 == 2:
                    h = s // 3
                    BPR = NBLK // 8
                    nc.sync.dma_start(
                        (out_t if skip_collective else partial_d)
                            [h * BPR:(h + 1) * BPR].rearrange("b p c -> p b c"),
                        part_sb[:, h * BPR * NW:(h + 1) * BPR * NW]
                            .rearrange("p (b c) -> p b c", b=BPR))
                if not skip_collective and rs_inloop and s >= 4 and (s - 4) % 3 == 0:
                    h = (s - 4) // 3
                    BPR = NBLK // 8
                    nc.gpsimd.collective_compute(
                        "ReduceScatter",
                        mybir.AluOpType.add,
                        replica_groups=[list(range(NCORE))],
                        ins=[partial_d[h * BPR:(h + 1) * BPR]],
                        outs=[rs_out[h]],
                    )

            if not skip_collective:
                BPR = NBLK // 8
                for h in ((7,) if rs_inloop else range(8)):
                    nc.gpsimd.collective_compute(
                        "ReduceScatter",
                        mybir.AluOpType.add,
                        replica_groups=[list(range(NCORE))],
                        ins=[partial_d[h * BPR:(h + 1) * BPR]],
                        outs=[rs_out[h]],
                    )

            # ---- Phase C: finalize ----
            if skip_collective:
                nc.compile_marker = None
            if not skip_collective:
              rs_flat = rs_out[:].rearrange("h p c -> (h p) c")
              for t in range(NBLK // NCORE):
                  fin16 = fpool.tile([128, NW], F16, tag="fin16")
                  nc.sync.dma_start(fin16[:], rs_flat[t * 128:(t + 1) * 128])
                  fin = fpool.tile([128, NW], F32, tag="fin")
                  nc.vector.tensor_copy(fin[:], fin16[:])
                  cnt = fpool.tile([128, 1], F32, tag="cnt")
                  nc.vector.tensor_scalar_max(cnt[:], fin[:, NCH:NW], 1.0)
                  rec = fpool.tile([128, 1], F32, tag="rec")
                  nc.vector.reciprocal(rec[:], cnt[:])
                  sc = fpool.tile([128, NCH], F32, tag="sc")
                  nc.vector.tensor_scalar_mul(sc[:], fin[:, :NCH], rec[:])
                  sc2 = fpool.tile([128, NCH], F32, tag="sc2")
                  nc.vector.tensor_add(sc2[:], sc[:], bias_sb[:])
                  og = fpool.tile([128, NCH], F32, tag="og")
                  nc.scalar.activation(og[:], sc2[:],
                                       mybir.ActivationFunctionType.Sigmoid)
                  nc.sync.dma_start(out_t[t * 128:(t + 1) * 128, :], og[:])

    nc.compile()
    return nc


def prep_inputs(class_capsules, W, b, point_idx, segment_ids, num_segments=NSEG):
    """Host-side sharding: returns in_maps (list of 8 dicts)."""
    assert int(num_segments) == NSEG
    grid = np.ascontiguousarray(class_capsules.reshape(GRID, D), np.float32)
    point_idx = np.asarray(point_idx, np.int64)
    segment_ids = np.asarray(segment_ids, np.int64)
    W = np.asarray(W, np.float32)
    b = np.asarray(b, np.float32)

    w_pack = np.zeros((128, 60), ml_dtypes.bfloat16)
    w20 = np.concatenate([W, np.zeros((D, 1), np.float32)], 1)  # [272, 19+pad]
    w_pack[:, 0:20] = w20[0:128].astype(ml_dtypes.bfloat16)
    w_pack[:, 20:40] = w20[128:256].astype(ml_dtypes.bfloat16)
    w_pack[0:16, 40:60] = w20[256:272].astype(ml_dtypes.bfloat16)

    iota = np.tile(np.arange(SEGBLK, dtype=np.float32), (128, 1)).astype(
        ml_dtypes.bfloat16)
    bias_rep = np.tile(b[None, :], (128, 1)).astype(np.float32)

    in_maps = []
    for k in range(NCORE):
        sel = (point_idx >= k * GPC) & (point_idx < (k + 1) * GPC)
        lidx = (point_idx[sel] - k * GPC).astype(np.int16)
        lseg = segment_ids[sel]          # still sorted ascending
        blk = (lseg >> 7).astype(np.int64)
        srel = (lseg & 127).astype(np.float32)
        counts = np.bincount(blk, minlength=NBLK)
        assert counts.max() <= CAP, f"core {k}: block count {counts.max()} > CAP"
        start = np.zeros(NBLK, np.int64)
        start[1:] = np.cumsum(counts)[:-1]
        rank = np.arange(lidx.size) - start[blk]
        pos = blk * CAP + rank

        idx_pad = np.zeros(NIDX, np.int16)
        srel_pad = np.full(NIDX, -1.0, np.float32)
        idx_pad[pos] = lidx
        srel_pad[pos] = srel

        segrel_arr = srel_pad.reshape(NCHUNK, 128).T.astype(ml_dtypes.bfloat16)
        idxw = np.empty((NSLICE, 128, IDX_PER_SLICE // 16), np.int16)
        for s in range(NSLICE):
            chunk = idx_pad[s * IDX_PER_SLICE:(s + 1) * IDX_PER_SLICE]
            idxw[s] = np.tile(chunk.reshape(-1, 16).T, (8, 1))

        gridT_k = np.ascontiguousarray(
            grid[k * GPC:(k + 1) * GPC].T).astype(ml_dtypes.bfloat16)

        in_maps.append({
            "gridT": gridT_k,
            "w_pack": w_pack,
            "idx": idxw,
            "segrel": np.ascontiguousarray(segrel_arr),
            "iota": iota,
            "bias": bias_rep,
        })
    return in_maps


def assemble(results):
    out = np.empty((NSEG, NCH), np.float32)
    for k in range(NCORE):
        r = results[k]["out"]  # [512, 19]: row i -> chunk h=i//64, j=i%64
        for h in range(8):
            out[4 * h * 128 + k * 64: 4 * h * 128 + (k + 1) * 64] =                 r[h * 64:(h + 1) * 64]
    return out


_NC_CACHE = {}


def kernel(class_capsules, W, b, point_idx, segment_ids, num_segments):
    """Full-input entry point: shard across 8 NeuronCores, run, reassemble."""
    from concourse.bass_utils import run_bass_kernel_spmd

    in_maps = prep_inputs(np.asarray(class_capsules), np.asarray(W),
                          np.asarray(b), np.asarray(point_idx),
                          np.asarray(segment_ids), int(num_segments))
    if "nc" not in _NC_CACHE:
        _NC_CACHE["nc"] = build_nc()
    res = run_bass_kernel_spmd(_NC_CACHE["nc"], in_maps, list(range(NCORE)))
    return assemble(res.results)



# revision 8
# speedup vs baseline: 1.3917x; 1.3917x over previous
"""CapsuleModel2 segment-reduce kernel for 8 TRN2 NeuronCores.

Math (per reference.py):
    feats  = class_capsules.reshape(65536, 272)[point_idx]        # [P, 272]
    sums   = segment_sum(feats, segment_ids, 4096)                # [4096, 272]
    counts = segment_sum(ones)                                    # [4096]
    out    = sigmoid((sums / max(counts,1)) @ W + b)              # [4096, 19]

Key identity used: (sums @ W) = segment_sum(feats @ W), so we project the
65536x272 grid down to 19 channels FIRST (on the PE), then gather 20-wide
rows (19 channels + a constant-1 count column) instead of 272-wide rows.

Distribution (table-sharded):
  - core k owns grid rows [k*8192, (k+1)*8192) and the points that hit them
  - each core computes partial sums over ALL 4096 segments via a one-hot
    matmul (exploits segment-sortedness: points are laid out per 128-segment
    block with padding, so each 128-point chunk stays inside one block)
  - a single ReduceScatter(add) gives core k the totals for segments
    [k*512, (k+1)*512); finalize: divide by counts, add bias, sigmoid

Schedule (v2):
  - points within each block are sorted by grid row; chunks are emitted
    j-major (chunk j of every block before chunk j+1 of any), so gather
    slice s only touches table rows < bound[s].  Each dma_gather reads
    table[0:bound[s]], which lets the tile scheduler overlap the gather
    phase with the projection that is still writing high table rows.
  - per-chunk matmul results are accumulated into an SBUF f32 accumulator
    (DVE add), since j-major interleaves blocks.
  - a dummy warmup ReduceScatter is issued at kernel start so the
    collective mesh setup cost lands during the projection phase; the real
    (single) ReduceScatter at the end is then cheap.
"""

import sys

for _p in ('/opt/trn_rl_repo',):
    if _p not in sys.path:
        sys.path.insert(0, _p)

import numpy as np
import ml_dtypes

import concourse.bacc as bacc
import concourse.bass as bass
import concourse.mybir as mybir
import concourse.tile as tile

BF16 = mybir.dt.bfloat16
F32 = mybir.dt.float32
I16 = mybir.dt.int16
F16 = mybir.dt.float16

NCORE = 8
GRID = 65536
GPC = GRID // NCORE          # 8192 grid rows per core
D = 272                      # capsule feature dim
NCH = 19                     # output channels
NW = NCH + 1                 # + count column
NSEG = 4096
SEGBLK = 128                 # segments per block
NBLK = NSEG // SEGBLK        # 32
CAP = 2304                   # padded points per (core, block); actual max 2181
CPB = CAP // 128             # 18 chunks per block
NCHUNK = NBLK * CPB          # 576 chunks per core
NIDX = NCHUNK * 128          # 73728 gather slots per core
NSLICE = 24
CHUNKS_PER_SLICE = NCHUNK // NSLICE   # 24
IDX_PER_SLICE = NIDX // NSLICE        # 3072
ELEM = 128                   # table row width (bf16) = 256B
SEG_PER_CORE = NSEG // NCORE  # 512
MTILE = 512                  # grid columns loaded per projection step


def build_nc(bounds=None):
    """bounds: per-slice table row bound (multiples of MTILE), len NSLICE."""
    if bounds is None:
        bounds = [GPC] * NSLICE
    nc = bacc.Bacc("TRN2", num_devices=NCORE, num_swdge_queues=4)

    gridT = nc.dram_tensor("gridT", [D, GPC], BF16, kind="ExternalInput")
    w_pack = nc.dram_tensor("w_pack", [128, 60], BF16, kind="ExternalInput")
    idx_in = nc.dram_tensor("idx", [NSLICE, 128, IDX_PER_SLICE // 16], I16,
                            kind="ExternalInput")
    segrel_in = nc.dram_tensor("segrel", [128, NCHUNK], BF16, kind="ExternalInput")
    iota_in = nc.dram_tensor("iota", [128, SEGBLK], BF16, kind="ExternalInput")
    bias_in = nc.dram_tensor("bias", [128, NCH], F32, kind="ExternalInput")
    out_t = nc.dram_tensor("out", [SEG_PER_CORE, NCH], F32, kind="ExternalOutput")

    table = nc.dram_tensor("table", [GPC, ELEM], BF16)
    partial_d = nc.dram_tensor("partial", [NBLK, 128, NW], F16)
    rs_out = nc.dram_tensor("rs_out", [NBLK // NCORE, 128, NW], F16)
    wu_in = nc.dram_tensor("wu_in", [8, 16], F16)
    wu_out = nc.dram_tensor("wu_out", [1, 16], F16)

    with tile.TileContext(nc) as tc:
        with (
            tc.tile_pool(name="const", bufs=1) as cpool,
            tc.tile_pool(name="grid", bufs=3) as gpool,
            tc.tile_pool(name="ptab", bufs=2, space="PSUM") as pt_pool,
            tc.tile_pool(name="tab", bufs=4) as tpool,
            tc.tile_pool(name="idxp", bufs=1) as ipool,
            tc.tile_pool(name="dst", bufs=6) as dpool,
            tc.tile_pool(name="oh", bufs=6) as opool,
            tc.tile_pool(name="pblk", bufs=6, space="PSUM") as pb_pool,
            tc.tile_pool(name="acc", bufs=1) as apool,
            tc.tile_pool(name="fin", bufs=2) as fpool,
        ):
            # ---- warmup collective: pay mesh setup during projection ----
            wu_sb = cpool.tile([8, 16], F16)
            nc.vector.memset(wu_sb[:], 0.0)
            nc.scalar.dma_start(wu_in[:], wu_sb[:])
            nc.gpsimd.collective_compute(
                "ReduceScatter",
                mybir.AluOpType.add,
                replica_groups=[list(range(NCORE))],
                ins=[wu_in[:]],
                outs=[wu_out[:]],
            )

            # constants
            w_sb = cpool.tile([128, 60], BF16)
            nc.sync.dma_start(w_sb[:], w_pack[:])
            segrel_sb = cpool.tile([128, NCHUNK], BF16)
            nc.sync.dma_start(segrel_sb[:], segrel_in[:])
            iota_sb = cpool.tile([128, SEGBLK], BF16)
            nc.sync.dma_start(iota_sb[:], iota_in[:])
            bias_sb = cpool.tile([128, NCH], F32)
            nc.sync.dma_start(bias_sb[:], bias_in[:])

            # f32 accumulator [seg-in-block, block*NW]
            acc = apool.tile([128, NBLK, NW], F32)
            nc.vector.memset(acc[:], 0.0)

            # ---- Phase A: projection -> table (row-major mt order) ----
            ksizes = [(0, 128), (128, 128), (256, 16)]
            for mt in range(GPC // MTILE):
                gt = gpool.tile([128, 3, MTILE], BF16, tag="gt")
                for t, (k0, kn) in enumerate(ksizes):
                    nc.sync.dma_start(
                        gt[:kn, t, :],
                        gridT[k0:k0 + kn, mt * MTILE:(mt + 1) * MTILE])
                tab = tpool.tile([128, MTILE // 128, ELEM], BF16, tag="tab")
                for ms in range(MTILE // 128):
                    psum = pt_pool.tile([128, NW], F32, tag="ptab")
                    for t, (k0, kn) in enumerate(ksizes):
                        nc.tensor.matmul(
                            out=psum[:],
                            lhsT=gt[:kn, t, ms * 128:(ms + 1) * 128],
                            rhs=w_sb[:kn, t * NW:(t + 1) * NW],
                            start=(t == 0), stop=(t == 2))
                    nc.vector.memset(tab[:, ms, NCH:], 0.0)
                    nc.vector.memset(tab[:, ms, NCH:NW], 1.0)
                    nc.vector.tensor_copy(tab[:, ms, :NCH], psum[:, :NCH])
                nc.scalar.dma_start(
                    table[mt * MTILE:(mt + 1) * MTILE]
                        .rearrange("(ms p) e -> p ms e", p=128),
                    tab[:])

            # ---- Phase B: gather + one-hot reduce (j-major chunks) ----
            idx_all = ipool.tile([128, NSLICE, IDX_PER_SLICE // 16], I16)
            nc.sync.dma_start(idx_all[:], idx_in[:].rearrange("s p c -> p s c"))
            for s in range(NSLICE):
                idx_sb = idx_all[:, s, :]
                dst = dpool.tile([128, CHUNKS_PER_SLICE, ELEM], BF16, tag="dst")
                nc.gpsimd.dma_gather(
                    dst[:], table[0:bounds[s]], idx_sb[:],
                    IDX_PER_SLICE, IDX_PER_SLICE, ELEM, single_packet=False,
                    queue_num=s % 4)
                oh = opool.tile([128, CHUNKS_PER_SLICE, SEGBLK], BF16, tag="oh")
                nc.vector.tensor_tensor(
                    out=oh[:],
                    in0=segrel_sb[:, s * CHUNKS_PER_SLICE:(s + 1) * CHUNKS_PER_SLICE]
                        .rearrange("p (c o) -> p c o", o=1).broadcast_to(
                            [128, CHUNKS_PER_SLICE, SEGBLK]),
                    in1=iota_sb[:].rearrange("p (o j) -> p o j", o=1).broadcast_to(
                            [128, CHUNKS_PER_SLICE, SEGBLK]),
                    op=mybir.AluOpType.is_equal)
                for g in range(CHUNKS_PER_SLICE):
                    gg = s * CHUNKS_PER_SLICE + g
                    blk = gg % NBLK         # j-major: chunk j of each block
                    psum_b = pb_pool.tile([128, NW], F32, tag="pblk")
                    nc.tensor.matmul(
                        out=psum_b[:],
                        lhsT=oh[:, g, :],
                        rhs=dst[:, g, :NW],
                        start=True, stop=True)
                    nc.vector.tensor_add(
                        acc[:, blk, :], acc[:, blk, :], psum_b[:])

            # ---- single ReduceScatter over all blocks ----
            part16 = apool.tile([128, NBLK, NW], F16)
            nc.vector.tensor_copy(part16[:], acc[:])
            nc.scalar.dma_start(
                partial_d[:].rearrange("b p c -> p b c"), part16[:])
            nc.gpsimd.collective_compute(
                "ReduceScatter",
                mybir.AluOpType.add,
                replica_groups=[list(range(NCORE))],
                ins=[partial_d[:]],
                outs=[rs_out[:]],
            )

            # ---- Phase C: finalize ----
            rs_flat = rs_out[:].rearrange("h p c -> (h p) c")
            for t in range(NBLK // NCORE):
                fin16 = fpool.tile([128, NW], F16, tag="fin16")
                nc.sync.dma_start(fin16[:], rs_flat[t * 128:(t + 1) * 128])
                fin = fpool.tile([128, NW], F32, tag="fin")
                nc.vector.tensor_copy(fin[:], fin16[:])
                cnt = fpool.tile([128, 1], F32, tag="cnt")
                nc.vector.tensor_scalar_max(cnt[:], fin[:, NCH:NW], 1.0)
                rec = fpool.tile([128, 1], F32, tag="rec")
                nc.vector.reciprocal(rec[:], cnt[:])
                sc = fpool.tile([128, NCH], F32, tag="sc")
                nc.vector.tensor_scalar_mul(sc[:], fin[:, :NCH], rec[:])
                sc2 = fpool.tile([128, NCH], F32, tag="sc2")
                nc.vector.tensor_add(sc2[:], sc[:], bias_sb[:])
                og = fpool.tile([128, NCH], F32, tag="og")
                nc.scalar.activation(og[:], sc2[:],
                                     mybir.ActivationFunctionType.Sigmoid)
                nc.sync.dma_start(out_t[t * 128:(t + 1) * 128, :], og[:])

    nc.compile()
    return nc


def prep_inputs(class_capsules, W, b, point_idx, segment_ids, num_segments=NSEG):
    """Host-side sharding: returns (in_maps, bounds)."""
    assert int(num_segments) == NSEG
    grid = np.ascontiguousarray(class_capsules.reshape(GRID, D), np.float32)
    point_idx = np.asarray(point_idx, np.int64)
    segment_ids = np.asarray(segment_ids, np.int64)
    W = np.asarray(W, np.float32)
    b = np.asarray(b, np.float32)

    w_pack = np.zeros((128, 60), ml_dtypes.bfloat16)
    w20 = np.concatenate([W, np.zeros((D, 1), np.float32)], 1)  # [272, 19+pad]
    w_pack[:, 0:20] = w20[0:128].astype(ml_dtypes.bfloat16)
    w_pack[:, 20:40] = w20[128:256].astype(ml_dtypes.bfloat16)
    w_pack[0:16, 40:60] = w20[256:272].astype(ml_dtypes.bfloat16)

    iota = np.tile(np.arange(SEGBLK, dtype=np.float32), (128, 1)).astype(
        ml_dtypes.bfloat16)
    bias_rep = np.tile(b[None, :], (128, 1)).astype(np.float32)

    # chunk (b, j) lives at slot range ((j*NBLK + b)*128, +128)
    in_maps = []
    # per-slice max row over all cores (for the in_ap bounds)
    slice_maxrow = np.zeros(NSLICE, np.int64)
    per_core = []
    for k in range(NCORE):
        sel = (point_idx >= k * GPC) & (point_idx < (k + 1) * GPC)
        lidx = (point_idx[sel] - k * GPC).astype(np.int64)
        lseg = segment_ids[sel]          # sorted ascending
        blk = (lseg >> 7).astype(np.int64)
        srel = (lseg & 127).astype(np.float32)
        counts = np.bincount(blk, minlength=NBLK)
        assert counts.max() <= CAP, f"core {k}: block count {counts.max()} > CAP"

        idx_pad = np.zeros(NIDX, np.int16)
        srel_pad = np.full(NIDX, -1.0, np.float32)
        for bb in range(NBLK):
            bsel = blk == bb
            bi = lidx[bsel]
            bs = srel[bsel]
            order = np.argsort(bi, kind="stable")   # row-sorted within block
            bi = bi[order]
            bs = bs[order]
            n = bi.size
            # positions: j-major layout
            pos_j = np.arange(n) // 128              # chunk j within block
            pos_i = np.arange(n) % 128
            pos = (pos_j * NBLK + bb) * 128 + pos_i
            idx_pad[pos] = bi.astype(np.int16)
            srel_pad[pos] = bs
            # track per-slice max row: chunk (bb, j) is global chunk j*NBLK+bb,
            # in slice (j*NBLK+bb)//CHUNKS_PER_SLICE
            for j in range(CPB):
                lo, hi = j * 128, min((j + 1) * 128, n)
                if lo >= n:
                    break
                sl = (j * NBLK + bb) // CHUNKS_PER_SLICE
                slice_maxrow[sl] = max(slice_maxrow[sl], int(bi[hi - 1]))
        per_core.append((idx_pad, srel_pad))

    # bounds: cumulative max over slices, rounded up to MTILE
    bounds = []
    run = 0
    for s in range(NSLICE):
        run = max(run, int(slice_maxrow[s]) + 1)
        bounds.append(min(GPC, ((run + MTILE - 1) // MTILE) * MTILE))

    for k in range(NCORE):
        idx_pad, srel_pad = per_core[k]
        segrel_arr = srel_pad.reshape(NCHUNK, 128).T.astype(ml_dtypes.bfloat16)
        idxw = np.empty((NSLICE, 128, IDX_PER_SLICE // 16), np.int16)
        for s in range(NSLICE):
            chunk = idx_pad[s * IDX_PER_SLICE:(s + 1) * IDX_PER_SLICE]
            idxw[s] = np.tile(chunk.reshape(-1, 16).T, (8, 1))

        gridT_k = np.ascontiguousarray(
            grid[k * GPC:(k + 1) * GPC].T).astype(ml_dtypes.bfloat16)

        in_maps.append({
            "gridT": gridT_k,
            "w_pack": w_pack,
            "idx": idxw,
            "segrel": np.ascontiguousarray(segrel_arr),
            "iota": iota,
            "bias": bias_rep,
        })
    return in_maps, tuple(bounds)


def assemble(results):
    out = np.empty((NSEG, NCH), np.float32)
    for k in range(NCORE):
        r = results[k]["out"]  # [512, 19]: rows = segs [512k, 512k+512)
        out[k * SEG_PER_CORE:(k + 1) * SEG_PER_CORE] = r
    return out


_NC_CACHE = {}


def kernel(class_capsules, W, b, point_idx, segment_ids, num_segments):
    """Full-input entry point: shard across 8 NeuronCores, run, reassemble."""
    from concourse.bass_utils import run_bass_kernel_spmd

    in_maps, bounds = prep_inputs(np.asarray(class_capsules), np.asarray(W),
                                  np.asarray(b), np.asarray(point_idx),
                                  np.asarray(segment_ids), int(num_segments))
    if _NC_CACHE.get("bounds") != bounds:
        _NC_CACHE["nc"] = build_nc(list(bounds))
        _NC_CACHE["bounds"] = bounds
    res = run_bass_kernel_spmd(_NC_CACHE["nc"], in_maps, list(range(NCORE)))
    return assemble(res.results)
